# revision 20
# baseline (speedup 1.0000x reference)
"""CQAttention Trainium2 kernel — fp16, software-pipelined.

Math (per batch b, D=128, Lc=1024, Lq=128):
    Ct = C[b].T  (Lc,D);  Qt = Q[b].T  (Lq,D)
    S[c,q] = (Ct[c]*w_m).Qt[q] + Ct[c].w_c + Qt[q].w_q
    S1 = softmax_q(S + qbias), S2 = softmax_c(S + cbias)
    A  = S1 @ Qt
    Bt = (S1 @ S2.T) @ Ct = S1 @ (S2.T @ Ct)       # W2 := S2.T@Ct is 128x128
    out[b] = concat([Ct, A, Ct*A, Ct*Bt], axis=1).T  # [4D, Lc]

Device computes blocks 1..3 (A, Ct*A, Ct*Bt) in fp16; block 0 is exactly
the input C[b], assembled on the host in fp32.  The rank-1 bias terms
s_q = Qt.w_q + qbias (fp32 input) and s_c = Ct.w_c + cbias (fp16 cols)
are tiny host matvecs.

Softmax factorization (shift-free; logits are O(1)):
    S1[c,q] = exp(s_m + s_q)[c,q] / z1[c],  z1[c] = sum_q exp(s_m+s_q)
    S2[c,q] = exp(s_m + s_c)[c,q] / z2[q],  z2[q] = sum_c exp(s_m+s_c)
exp(s_c) is applied multiplicatively on the w2 GEMM's moving operand
(rhsB = [CT|1] * esc), so the e2 exps run biasless as 4x256-col acts.

Pipeline: batch b's work is split into stages emitted two iterations
apart so every engine's in-order queue interleaves consecutive batches:
  iter k:  st/e1/z1/at/s0 of batch k-1,  w2/bt/out of batch k-2,
           load of batch k.
The output DMA is issued from SP after the loads so it never
head-of-line blocks the input stream.
"""

import warnings

warnings.filterwarnings("ignore")

import numpy as np

B, D, LC, LQ = 64, 128, 1024, 128
NT = 8  # c-tiles per batch
NCORES = 8
NB = B // NCORES  # batches per core
NEG16 = -30000.0  # fp16-representable "minus infinity" for mask biases
NIN = LC + LQ + NT  # input row: cb | qb | scb8

CFG = {
    "pipe": 2,        # stage skew
}

_CACHE = {}


def _build_nc(reps=1):
    import concourse.bass as bass
    import concourse.mybir as mybir
    import concourse.tile as tile
    from concourse import bacc

    F32 = mybir.dt.float32
    F16 = mybir.dt.float16
    AF = mybir.ActivationFunctionType

    nc = bacc.Bacc("TRN2", target_bir_lowering=False, debug=False,
                   num_devices=NCORES)

    In16 = nc.dram_tensor("In16", [NB, D, NIN], F16, kind="ExternalInput")
    SQ32 = nc.dram_tensor("SQ32", [NB, D, 1], F32, kind="ExternalInput")
    W16 = nc.dram_tensor("W16", [3, D], F16, kind="ExternalInput")
    Out = nc.dram_tensor("Out", [NB, D, 3, LC], F16, kind="ExternalOutput")

    with tile.TileContext(nc) as tc:
        with tc.tile_pool(name="const", bufs=1) as constp, \
             tc.tile_pool(name="io", bufs=4) as iop, \
             tc.tile_pool(name="sb", bufs=4) as sb, \
             tc.tile_pool(name="sm", bufs=4) as sm, \
             tc.tile_pool(name="ps_big", bufs=3, space="PSUM") as ps_big, \
             tc.tile_pool(name="ps_s0", bufs=2, space="PSUM") as ps_s0:

            # ---- constants ----
            ones16 = constp.tile([D, D], F16)
            nc.gpsimd.memset(ones16[:], 1.0)
            wm = constp.tile([D, 1], F16)
            nc.sync.dma_start(wm[:], W16[2, :, None])

            def batch_phases(b):
                st = {}

                def ph_load():
                    inb = iop.tile([D, NIN], F16, tag="inb", name=f"inb{b}")
                    st["inb"] = inb
                    nc.sync.dma_start(inb[:], In16[b])
                    sq32 = sm.tile([D, 1], F32, tag="sq32", name=f"sq32{b}")
                    st["sq32"] = sq32
                    nc.sync.dma_start(sq32[:], SQ32[b])

                def ph_transp():
                    # one transpose covers the 8 C-tiles and Q (Act-issued,
                    # emitted at end of iteration so it never blocks acts)
                    inb = st["inb"]
                    rhsT = sb.tile([D, NT + 1, D], F16, tag="rhsT",
                                   name=f"rhsT{b}")
                    st["rhsT"] = rhsT
                    nc.scalar.dma_start_transpose(rhsT[:], inb[:, 0:LC + LQ])

                def ph_st():
                    inb, rhsT = st["inb"], st["rhsT"]
                    cb = inb[:, 0:LC]
                    qb = inb[:, LC:LC + LQ]
                    esc = sm.tile([D, NT], F16, tag="esc", name=f"esc{b}")
                    nc.scalar.activation(esc[:], inb[:, LC + LQ:NIN], AF.Exp)
                    qww = sm.tile([D, LQ], F16, tag="qww", name=f"qww{b}")
                    st["qww"] = qww
                    nc.gpsimd.tensor_mul(qww[:], qb,
                                         wm[:].to_broadcast((D, LQ)))
                    rhsB = sb.tile([D, NT, 132], F16, tag="rhsB",
                                   name=f"rhsB{b}")
                    st["rhsB"] = rhsB
                    nc.gpsimd.tensor_mul(
                        rhsB[:, :, 0:128], rhsT[:, 0:NT, :],
                        esc[:, :, None].broadcast_to((D, NT, D)))
                    nc.gpsimd.tensor_copy(rhsB[:, :, 128:129],
                                          esc[:, :, None])
                    p_st = ps_big.tile([D, LC], F32, tag="big", name=f"pst{b}")
                    st["p_st"] = p_st
                    for h in range(2):
                        nc.tensor.matmul(p_st[:, 512 * h:512 * (h + 1)],
                                         qww[:],
                                         cb[:, 512 * h:512 * (h + 1)],
                                         start=True, stop=True)

                def ph_e1():
                    p_st = st["p_st"]
                    e1 = sb.tile([D, LC], F16, tag="e1", name=f"e1_{b}")
                    st["e1"] = e1
                    nc.scalar.activation(e1[:], p_st[:], AF.Exp,
                                         bias=st["sq32"][:])

                def ph_z1at():
                    e1 = st["e1"]
                    qt = st["rhsT"][:, NT, :]
                    inb = st["inb"]
                    cb = inb[:, 0:LC]
                    p_z1 = ps_big.tile([D, LC], F32, tag="big", name=f"pz1{b}")
                    for h in range(2):
                        nc.tensor.matmul(p_z1[:, 512 * h:512 * (h + 1)],
                                         ones16[:],
                                         e1[:, 512 * h:512 * (h + 1)],
                                         start=True, stop=True)
                    r1 = sb.tile([D, LC], F32, tag="r1", name=f"r1_{b}")
                    st["r1"] = r1
                    nc.vector.reciprocal_approx_fast(r1[:], p_z1[:])
                    p_at = ps_big.tile([D, LC], F32, tag="big", name=f"pat{b}")
                    for h in range(2):
                        nc.tensor.matmul(p_at[:, 512 * h:512 * (h + 1)], qt,
                                         e1[:, 512 * h:512 * (h + 1)],
                                         start=True, stop=True)
                    ob = iop.tile([D, 3, LC], F16, tag="ob", name=f"ob{b}")
                    st["ob"] = ob
                    nc.vector.tensor_mul(ob[:, 0, :], p_at[:], r1[:])
                    nc.gpsimd.tensor_mul(ob[:, 1, :], ob[:, 0, :], cb[:])

                def ph_s0():
                    inb, qww = st["inb"], st["qww"]
                    cb = inb[:, 0:LC]
                    e2 = sb.tile([D, NT, D], F16, tag="e2", name=f"e2_{b}")
                    st["e2"] = e2
                    for g in range(4):
                        p_s0 = ps_s0.tile([D, 2, 256], F32, tag="s0",
                                          name=f"ps0{b}_{g}")
                        for j in range(2):
                            ct = 2 * g + j
                            nc.tensor.matmul(p_s0[:, j, 0:128],
                                             cb[:, ct * D:(ct + 1) * D],
                                             qww[:],
                                             start=True, stop=True)
                        nc.scalar.activation(e2[:, 2 * g:2 * g + 2, :],
                                             p_s0[:, :, 0:128], AF.Exp)

                def ph_w2():
                    e2, rhsB = st["e2"], st["rhsB"]
                    p_w2 = ps_s0.tile([D, 2, 256], F32, tag="s0",
                                      name=f"pw2{b}")
                    for ct in range(NT):
                        nc.tensor.matmul(p_w2[:, 0, 0:129], e2[:, ct, :],
                                         rhsB[:, ct, 0:129],
                                         start=(ct == 0), stop=(ct == NT - 1))
                    r2 = sm.tile([D, 1], F32, tag="r2", name=f"r2_{b}")
                    nc.vector.reciprocal(r2[:], p_w2[:, 0, 128:129])
                    w2 = sb.tile([D, D], F16, tag="w2sb", name=f"w2_{b}")
                    st["w2"] = w2
                    nc.scalar.activation(w2[:], p_w2[:, 0, 0:128], AF.Copy,
                                         scale=r2[:])

                def ph_bt():
                    e1, w2, r1 = st["e1"], st["w2"], st["r1"]
                    inb, ob = st["inb"], st["ob"]
                    cb = inb[:, 0:LC]
                    p_bt = ps_big.tile([D, LC], F32, tag="big", name=f"pbt{b}")
                    for h in range(2):
                        nc.tensor.matmul(p_bt[:, 512 * h:512 * (h + 1)], w2[:],
                                         e1[:, 512 * h:512 * (h + 1)],
                                         start=True, stop=True)
                    t3 = sb.tile([D, LC], F16, tag="t3", name=f"t3_{b}")
                    nc.vector.tensor_mul(t3[:], p_bt[:], r1[:])
                    nc.gpsimd.tensor_mul(ob[:, 2, :], t3[:], cb[:])

                def emit_out():
                    nc.gpsimd.dma_start(Out[b], st["ob"][:])

                st["phases"] = dict(load=ph_load, transp=ph_transp,
                                    st=ph_st, e1=ph_e1,
                                    z1at=ph_z1at, s0=ph_s0, w2=ph_w2,
                                    bt=ph_bt, out=emit_out)
                return st

            order = [bb for _ in range(reps) for bb in range(NB)]
            N = len(order)
            P = {}
            for k in range(N + 2):
                if k < N:
                    P[k] = batch_phases(order[k])
                    P[k]["phases"]["load"]()
                if 0 <= k - 1 < N:
                    P[k - 1]["phases"]["st"]()
                if 0 <= k - 2 < N:
                    P[k - 2]["phases"]["w2"]()
                    P[k - 2]["phases"]["bt"]()
                if 0 <= k - 1 < N:
                    P[k - 1]["phases"]["e1"]()
                    P[k - 1]["phases"]["z1at"]()
                    P[k - 1]["phases"]["s0"]()
                if 0 <= k - 2 < N:
                    P[k - 2]["phases"]["out"]()
                if k < N:
                    P[k]["phases"]["transp"]()

    nc.compile()
    return nc


def _prep_inmaps(C, Q, cmask, qmask, w):
    C64 = np.asarray(C, np.float64)
    Q64 = np.asarray(Q, np.float64)
    w64 = np.asarray(w, np.float64)
    wq, wc = w64[:D], w64[D:2 * D]
    C16 = C64.astype(np.float16)                          # [B, D, LC]
    Q16 = Q64.astype(np.float16)                          # [B, D, LQ]
    qbias = (1.0 - np.asarray(qmask, np.float64)) * NEG16
    cbias = (1.0 - np.asarray(cmask, np.float64)) * NEG16
    sq32 = np.ascontiguousarray(
        (np.einsum("bdq,d->bq", Q64, wq) + qbias)
        .astype(np.float32)[:, :, None])                  # [B, D(q), 1]
    scb = (np.einsum("bdc,d->bc", C64, wc) + cbias).astype(np.float16)
    scb8 = np.ascontiguousarray(
        scb.reshape(B, NT, D).transpose(0, 2, 1))         # [B, D, NT]
    in16 = np.concatenate([C16, Q16, scb8], axis=2)       # [B, D, NIN]
    in16 = np.ascontiguousarray(in16)
    w16 = np.ascontiguousarray(w64.reshape(3, D).astype(np.float16))
    in_maps = []
    for k in range(NCORES):
        s = slice(k * NB, (k + 1) * NB)
        in_maps.append({"In16": in16[s], "SQ32": sq32[s], "W16": w16})
    return in_maps


def _run(C, Q, cmask, qmask, w, trace=False):
    from concourse.bass_utils import run_bass_kernel_spmd

    key = (tuple(sorted(CFG.items())), 1)
    if key not in _CACHE:
        _CACHE[key] = _build_nc()
    nc = _CACHE[key]
    in_maps = _prep_inmaps(C, Q, cmask, qmask, w)
    res = run_bass_kernel_spmd(nc, in_maps, core_ids=list(range(NCORES)),
                               trace=trace)
    dev = np.concatenate([res.results[k]["Out"] for k in range(NCORES)],
                         axis=0)                          # [B, D, 3, LC] fp16
    return dev, res


def _assemble(dev, C):
    """dev: [n, D, 3, LC] fp16 device blocks; C: [n, D, LC] input."""
    n = dev.shape[0]
    out = np.empty((n, 4 * D, LC), np.float32)
    out[:, 0:D, :] = np.asarray(C, np.float32)            # block 0 exact
    out[:, D:4 * D, :] = dev.transpose(0, 2, 1, 3).reshape(
        n, 3 * D, LC).astype(np.float32)
    return out


def kernel(C, Q, cmask, qmask, w):
    dev, _ = _run(C, Q, cmask, qmask, w, trace=False)
    return _assemble(dev, C)
